# revision 7
# baseline (speedup 1.0000x reference)
"""ChannelFusionModule TRN2 kernel: batch-sharded, collective-free, fp16-resident.

Sharding: core k owns samples [2k, 2k+2) with ALL channels of both tensors,
so the pooled reduction is core-local -- no collectives, no cross-core barrier.
Weights (w1.T chunk-major, w2.T) are replicated (host transpose = data prep).

A full sample (33.6 MB f32) exceeds SBUF, so resident tiles are fp16. Engine
assignment is driven by measured DVE perf modes (scalar_tensor_tensor and
tensor_scalar+accum run 1x = 10.4 us/half; tensor_scalar hits 4x = 2.4 us;
tensor_tensor add hits 2x):
  - loads: [128, 4096] f32 quarters on the sync (HWDGE) queue -- pure loads,
    nothing else, so the queue never head-of-line blocks,
  - cast+rowsum: ACT Identity(accum_out) f32->fp16, one op per quarter,
  - MLP: PE matmuls; relu+1/HW folded into one DVE tensor_scalar (max, mult),
  - scale: DVE ts_mul xf*=s_f, ts_mul xm*=s_m (last sample: xm muls on ACT to
    shorten the exposed tail), then DVE tensor_tensor add in place,
  - stores: [128, 8192] fp16 halves on the gpsimd (SWDGE) queue, which is
    otherwise idle, so store sem-waits never block loads or compute.

HBM traffic/core: 67.1 MB f32 read + 16.8 MB fp16 write = 83.9 MB
(~234 us at the 358 GB/s per-core HBM limit). fp16 quantization of resident
data and output gives rel err ~3e-4 (gate: 2e-2).
"""

from contextlib import ExitStack

import numpy as np

import concourse.bacc as bacc
import concourse.tile as tile
from concourse import mybir
from concourse.bass import ts
from concourse.bass_utils import run_bass_kernel_spmd

N_CORES = 8
B, C, H, W = 16, 256, 128, 128
HW = H * W                    # 16384
P = 128
BL = B // N_CORES             # local samples per core (2)
NCH = 2 * C // P              # pooled chunks (4): fft c0, fft c1, multi c0, multi c1
R = C // 4                    # hidden dim (64)
HF = HW // 2                  # 8192: resident half-tile free extent
QF = HW // 4                  # 4096: load quarter free extent

F32 = mybir.dt.float32
F16 = mybir.dt.float16


def _emit(ctx, tc, nc, fft, mlt, w1t, w2t, out):
    # [b, (c p), (h hq hh), w] -> [b, c, p, h, hq, (hh w)]: chunk c of 128
    # channels on partitions, spatial split into 2 halves x 2 quarters
    fftv = fft.rearrange("b (c p) (h hq hh) w -> b c p h hq (hh w)", c=2, h=2, hq=2)
    mltv = mlt.rearrange("b (c p) (h hq hh) w -> b c p h hq (hh w)", c=2, h=2, hq=2)
    outv = out.rearrange("b (c p) (h hh) w -> b c p h (hh w)", c=2, h=2)

    consts = ctx.enter_context(tc.tile_pool(name="consts", bufs=1))
    tpool = ctx.enter_context(tc.tile_pool(name="tpool", bufs=4))
    rpool = ctx.enter_context(tc.tile_pool(name="rpool", bufs=8))
    small = ctx.enter_context(tc.tile_pool(name="small", bufs=2))
    ps_h = ctx.enter_context(tc.tile_pool(name="ps_h", bufs=2, space="PSUM"))
    ps_a = ctx.enter_context(tc.tile_pool(name="ps_a", bufs=4, space="PSUM"))

    # ---- replicated weights (emitted after the first data loads begin) ----
    w1t_sb = consts.tile([P, NCH, R], F32)
    w2t_sb = consts.tile([R, 2 * C], F32)

    for b in range(BL):
        last = b == BL - 1
        # ---- load quarters + cast-to-fp16 + row-sums ----
        # Cast engine: ACT primarily; DVE takes every other quarter whenever it
        # is otherwise idle (all of sample 0; back half of the last sample) so
        # the cast stream never gates the 4.9 us/quarter DMA cadence and no
        # cast backlog is left after the final load.
        # Per-engine partials tiles: a tile written by two engines serializes
        # them (tile-granular dependency tracking), so ACT and DVE each accum
        # into their own zeroed [128, 16]; reduced and summed at the end.
        # Likewise each resident half has exactly ONE cast engine (h parity).
        partials_a = small.tile([P, 4 * NCH], F32, tag="pa", name="pa")
        partials_d = small.tile([P, 4 * NCH], F32, tag="pd", name="pd")
        nc.scalar.memzero(partials_a)
        nc.vector.memset(partials_d, 0.0)
        xs = {}
        for u in range(4 * NCH):
            t, c, h, hq = u // 8, (u // 4) % 2, (u // 2) % 2, u % 2
            src = (fftv, mltv)[t]
            tr = tpool.tile([P, QF], F32, tag="T", name="tr")
            nc.sync.dma_start(out=tr, in_=src[b, c, :, h, hq, :])
            if b == 0 and u == 1:
                nc.sync.dma_start(out=w1t_sb, in_=w1t)
                nc.sync.dma_start(out=w2t_sb, in_=w2t)
            if hq == 0:
                xs[t, c, h] = rpool.tile([P, HF], F16, tag="R", name="x")
            dve_cast = (h == 1) and (b == 0 or u >= 8)
            if dve_cast:
                nc.vector.tensor_scalar(
                    out=xs[t, c, h][:, ts(hq, QF)],
                    in0=tr,
                    scalar1=1.0,
                    scalar2=0.0,
                    op0=mybir.AluOpType.mult,
                    op1=mybir.AluOpType.add,
                    accum_out=partials_d[:, u : u + 1],
                )
            else:
                nc.scalar.activation(
                    out=xs[t, c, h][:, ts(hq, QF)],
                    in_=tr,
                    func=mybir.ActivationFunctionType.Identity,
                    accum_out=partials_a[:, u : u + 1],
                )

        # ---- pooled chunks + tiny MLP (PE) ----
        pooled = small.tile([P, NCH], F32, tag="pooled", name="pooled")
        pooled_d = small.tile([P, NCH], F32, tag="pooled_d", name="pooled_d")
        nc.vector.reduce_sum(
            out=pooled,
            in_=partials_a.rearrange("p (k q) -> p k q", q=4),
            axis=mybir.AxisListType.X,
        )
        nc.vector.reduce_sum(
            out=pooled_d,
            in_=partials_d.rearrange("p (k q) -> p k q", q=4),
            axis=mybir.AxisListType.X,
        )
        nc.vector.tensor_tensor(
            out=pooled, in0=pooled, in1=pooled_d, op=mybir.AluOpType.add
        )
        hp = ps_h.tile([R, 1], F32, tag="hp", name="hp")
        for k in range(NCH):
            nc.tensor.matmul(
                hp,
                lhsT=w1t_sb[:, k, :],
                rhs=pooled[:, k : k + 1],
                start=(k == 0),
                stop=(k == NCH - 1),
            )
        # hT = relu(hp) / HW  (fold the mean's 1/HW here; sigmoid doesn't commute)
        hT = small.tile([R, 1], F32, tag="hT", name="hT")
        nc.vector.tensor_scalar(
            out=hT,
            in0=hp,
            scalar1=0.0,
            scalar2=1.0 / HW,
            op0=mybir.AluOpType.max,
            op1=mybir.AluOpType.mult,
        )
        s = small.tile([P, NCH], F32, tag="s", name="s")
        aps = ps_a.tile([P, NCH], F32, tag="aps", name="aps")
        for k in range(NCH):
            nc.tensor.matmul(
                aps[:, k : k + 1],
                lhsT=w2t_sb[:, ts(k, P)],
                rhs=hT,
                start=True,
                stop=True,
                skip_group_check=True,
            )
        nc.scalar.activation(
            out=s, in_=aps, func=mybir.ActivationFunctionType.Sigmoid
        )

        # ---- scale + store (stores ride the idle gpsimd SWDGE queue) ----
        # Last sample's tail is latency-critical: ACT takes the xm muls of the
        # last two units (running ahead concurrently) while DVE burns through
        # tsf/TT, so the TT chain is never ACT-blocked.
        for c in range(2):
            for h in range(2):
                xf, xm = xs[0, c, h], xs[1, c, h]
                s_f, s_m = s[:, c : c + 1], s[:, 2 + c : 3 + c]
                nc.vector.tensor_scalar_mul(out=xf, in0=xf, scalar1=s_f)
                if last and c * 2 + h >= 2:
                    nc.scalar.mul(out=xm, in_=xm, mul=s_m)
                else:
                    nc.vector.tensor_scalar_mul(out=xm, in0=xm, scalar1=s_m)
                nc.vector.tensor_tensor(
                    out=xf, in0=xf, in1=xm, op=mybir.AluOpType.add
                )
                nc.gpsimd.dma_start(out=outv[b, c, :, h, :], in_=xf)


def build_nc():
    nc = bacc.Bacc("TRN2", target_bir_lowering=False, debug=False, num_devices=N_CORES)
    fft = nc.dram_tensor("fft_features", [BL, C, H, W], F32, kind="ExternalInput").ap()
    mlt = nc.dram_tensor("multi_features", [BL, C, H, W], F32, kind="ExternalInput").ap()
    w1t = nc.dram_tensor("w1t", [P, NCH, R], F32, kind="ExternalInput").ap()
    w2t = nc.dram_tensor("w2t", [R, 2 * C], F32, kind="ExternalInput").ap()
    out = nc.dram_tensor("out", [BL, C, H, W], F16, kind="ExternalOutput").ap()

    with tile.TileContext(nc) as tc:
        with ExitStack() as ctx:
            _emit(ctx, tc, nc, fft, mlt, w1t, w2t, out)
    nc.compile()
    return nc


_NC_CACHE = None


def _get_nc():
    global _NC_CACHE
    if _NC_CACHE is None:
        _NC_CACHE = build_nc()
    return _NC_CACHE


def run(inputs, **spmd_kwargs):
    fft = np.asarray(inputs["fft_features"], dtype=np.float32)
    mlt = np.asarray(inputs["multi_features"], dtype=np.float32)
    w1 = np.asarray(inputs["w1"], dtype=np.float32)
    w2 = np.asarray(inputs["w2"], dtype=np.float32)
    assert fft.shape == (B, C, H, W), fft.shape

    # host data prep (transposes only): w1.T chunk-major [128, 4, 64], w2.T
    w1t = np.ascontiguousarray(w1.T.reshape(NCH, P, R).transpose(1, 0, 2))
    w2t = np.ascontiguousarray(w2.T)
    nc = _get_nc()
    in_maps = []
    for k in range(N_CORES):
        sl = slice(k * BL, (k + 1) * BL)
        in_maps.append(
            {
                "fft_features": np.ascontiguousarray(fft[sl]),
                "multi_features": np.ascontiguousarray(mlt[sl]),
                "w1t": w1t,
                "w2t": w2t,
            }
        )
    res = run_bass_kernel_spmd(nc, in_maps, core_ids=list(range(N_CORES)), **spmd_kwargs)
    outp = np.concatenate([np.asarray(r["out"], dtype=np.float32) for r in res.results])
    return outp, res


def kernel(**inputs) -> np.ndarray:
    outp, _ = run(inputs)
    return outp
